# revision 32
# baseline (speedup 1.0000x reference)
"""GraphSAGE-style 3-layer GNN (mean aggregation) on 8 Trainium2 NeuronCores.

Strategy (dst-sharded graph parallelism):
- Nodes (and their incoming edges) are sharded across 8 cores: core d owns
  dst nodes [d*6250, (d+1)*6250).
- Host preprocessing sorts each core's edges by (dst node-tile, src>=SPLIT,
  src) and packs them into 128-edge chunks. Per tile t the chunk counts
  (Mlo[t], Mhi[t]) are maxed over cores so all 8 cores share one program;
  pad slots get dloc=200 (matches no one-hot column, so they contribute 0).
- Aggregation on device: per tile a binary one-hot block
  OH[e, n] = (dloc[e] == n) is built on the DVE from an iota tile, and
  sumT[c, n] += G[e, c].T @ OH[e, n] accumulates in PSUM on the PE. The
  1/deg mean scaling is folded into the PSUM->SBUF copy (tensor_tensor mult
  with a replicated inv-degree row).
- Layer-1 edge source features are pre-gathered on the host and streamed
  sequentially; layers 2/3 gather their source features from a replicated
  DRAM table with batched gpsimd dma_gather calls (2 per tile: src<SPLIT
  and src>=SPLIT bases, int16 relative indices).
- The replicated table is refreshed between layers with an AllGather
  (halo exchange) over the 8 cores into per-layer Shared DRAM buffers.
- Dense transforms run in transposed layout: hT = Wl.T @ meanT + Wr.T @ xT,
  bias add + ReLU on the DVE, then PE-transposes produce row-major h for the
  next layer's gather table.
Compute dtype: bf16 inputs with fp32 PSUM accumulation.
"""
import os
import numpy as np
import ml_dtypes

import concourse.bass as bass
import concourse.bacc as bacc
import concourse.mybir as mybir
import concourse.tile as tile
from concourse.bass_utils import run_bass_kernel_spmd

N = 50000
E = 800000
C1 = 128
HID = 256
OUT = 15
NCORES = 8
NP = N // NCORES          # 6250 own nodes per core
P = 128
TILES = (NP + P - 1) // P  # 49 node tiles per core
NPAD = TILES * P           # 6272
KB = 6                     # L1 staged-gather chunks per DMA
SPLIT = 25088              # low/high src base split for int16 gather indices

bf16 = mybir.dt.bfloat16
f32 = mybir.dt.float32
_bf = ml_dtypes.bfloat16
ABLATE = set(os.environ.get("ABLATE", "").split(",")) - {""}  # timing experiments
GVAR = os.environ.get("GVAR", "gant2")  # "gant2" | "bind" | "gant" | "ind"


def _preprocess(x, edge_index):
    """Sort/pad edges into per-tile (lo, hi) chunk blocks shared across cores."""
    src = np.ascontiguousarray(edge_index[0]).astype(np.int64)
    dst = np.ascontiguousarray(edge_index[1]).astype(np.int64)
    deg = np.bincount(dst, minlength=N)
    invdeg = (1.0 / np.maximum(deg, 1)).astype(np.float32)

    core = dst // NP
    tilei = (dst - core * NP) // P
    ishi = (src >= SPLIT).astype(np.int64)
    gkey = (core * TILES + tilei) * 2 + ishi
    order = np.lexsort((src, gkey))
    sg = gkey[order]
    ssrc = src[order]
    sdst = dst[order]
    NG = NCORES * TILES * 2
    starts = np.searchsorted(sg, np.arange(NG))
    counts = np.diff(np.append(starts, E)).reshape(NCORES, TILES, 2)
    mlo = np.ceil(counts[:, :, 0] / P).astype(np.int64).max(axis=0)  # [TILES]
    mhi = np.ceil(counts[:, :, 1] / P).astype(np.int64).max(axis=0)
    mtot = mlo + mhi
    assert mtot.min() > 0
    chunkoff = np.zeros(TILES + 1, np.int64)
    chunkoff[1:] = np.cumsum(mtot)
    NCH = int(chunkoff[-1])

    # slot base for each (tile, lo/hi) block, per core (same for all cores)
    blk_slot0 = np.zeros((TILES, 2), np.int64)
    blk_slot0[:, 0] = chunkoff[:-1] * P
    blk_slot0[:, 1] = (chunkoff[:-1] + mlo) * P

    # per-core flattened slot grids [NCH*128]
    rank = np.arange(E) - starts[sg]
    tile_of = (sg // 2) % TILES
    hi_of = sg % 2
    slot_in_core = blk_slot0[tile_of, hi_of] + rank
    core_of = sg // (2 * TILES)

    srcg = np.zeros((NCORES, NCH * P), np.int64)
    # pad defaults: lo blocks gather row 0, hi blocks gather row SPLIT
    for t in range(TILES):
        srcg[:, blk_slot0[t, 1]:blk_slot0[t, 1] + mhi[t] * P] = SPLIT
    dlocg = np.full((NCORES, NCH * P), 200.0, np.float32)
    srcg[core_of, slot_in_core] = ssrc
    dlocg[core_of, slot_in_core] = sdst - core_of * NP - tile_of * P

    x_bf = x.astype(_bf)
    rel = srcg - (srcg >= SPLIT) * SPLIT
    assert rel.max() < 32768
    per_core = []
    for d in range(NCORES):
        # int16 gather indices packed [16, n/16] per call block, replicated x8
        relc = rel[d].astype(np.int16).reshape(NCH, P)
        packed = relc.reshape(NCH * 8, 16).T            # [16, NCH*8]
        gidx = np.ascontiguousarray(np.tile(packed, (8, 1)))  # [128, NCH*8]
        idx32 = np.ascontiguousarray(
            srcg[d].reshape(NCH, P).T).astype(np.int32)  # [128, NCH]
        # half-row (256 B) gather indices: pairs (2r, 2r+1) per edge
        idx32h = np.empty((P, 2 * NCH), np.int32)
        idx32h[:, 0::2] = 2 * idx32
        idx32h[:, 1::2] = 2 * idx32 + 1
        dloc = np.ascontiguousarray(
            dlocg[d].reshape(NCH, P).T).astype(_bf)      # [128, NCH]
        invd = np.zeros(NPAD, np.float32)
        invd[:NP] = invdeg[d * NP:(d + 1) * NP]
        invd = np.ascontiguousarray(
            np.tile(invd[None, :], (P, 1))).astype(_bf)  # [128, NPAD]
        # L1 pre-gathered edge features [128, NCH, C1]
        xg1 = np.ascontiguousarray(
            x_bf[srcg[d].reshape(NCH, P)].transpose(1, 0, 2))
        # own transposed features [128, NPAD]
        xT = np.zeros((C1, NPAD), np.float32)
        xT[:, :NP] = x[d * NP:(d + 1) * NP].T
        per_core.append(dict(gidx=gidx, idx32=idx32, idx32h=idx32h, dloc=dloc,
                             invd=invd, xg1=xg1,
                             xT=np.ascontiguousarray(xT).astype(_bf)))
    return per_core, tuple(int(v) for v in mlo), tuple(int(v) for v in mhi)


def _build(nc: bass.Bass, mlos, mhis, gvar=None):
    GVAR = gvar if gvar is not None else globals()["GVAR"]
    mtot = [a + b for a, b in zip(mlos, mhis)]
    choff = [0]
    for m in mtot:
        choff.append(choff[-1] + m)
    NCH = choff[-1]
    MMAX = max(mtot)
    # ---- I/O ----
    gidx_d = nc.dram_tensor("gidx", [P, NCH * 8], mybir.dt.int16, kind="ExternalInput")
    idx32_d = nc.dram_tensor("idx32", [P, NCH], mybir.dt.int32, kind="ExternalInput")
    idx32h_d = nc.dram_tensor("idx32h", [P, 2 * NCH], mybir.dt.int32,
                              kind="ExternalInput")
    dloc_d = nc.dram_tensor("dloc", [P, NCH], bf16, kind="ExternalInput")
    invd_d = nc.dram_tensor("invd", [P, NPAD], bf16, kind="ExternalInput")
    xg1_d = nc.dram_tensor("xg1", [P, NCH, C1], bf16, kind="ExternalInput")
    xT_d = nc.dram_tensor("xT", [P, NPAD], bf16, kind="ExternalInput")
    iota_d = nc.dram_tensor("iota", [P, P], bf16, kind="ExternalInput")
    wl_d = {}
    wr_d = {}
    for l, cin in ((1, C1), (2, HID), (3, HID)):
        wl_d[l] = nc.dram_tensor(f"Wl{l}", [cin, HID], bf16, kind="ExternalInput")
        wr_d[l] = nc.dram_tensor(f"Wr{l}", [cin, HID], bf16, kind="ExternalInput")
    bl_d = nc.dram_tensor("bl", [P, 6], f32, kind="ExternalInput")       # [:, 2(l-1)+j]
    wo_d = nc.dram_tensor("Wo", [HID, OUT], bf16, kind="ExternalInput")
    bo_d = nc.dram_tensor("bo", [1, OUT], f32, kind="ExternalInput")
    out_d = nc.dram_tensor("out", [NP, OUT], f32, kind="ExternalOutput")

    KH = HID // P  # 2 halves of the hidden dim
    nblocks = [(b, min(512, NPAD - b)) for b in range(0, NPAD, 512)]

    with tile.TileContext(nc) as tc:
        with (
            tc.tile_pool(name="const", bufs=1) as cst,
            tc.tile_pool(name="feat", bufs=1) as featp,
            tc.tile_pool(name="g", bufs=24) as gp,
            tc.tile_pool(name="gb", bufs=3) as gbp,
            tc.tile_pool(name="gb1", bufs=1) as gb1p,
            tc.tile_pool(name="oh", bufs=4) as ohp,
            tc.tile_pool(name="stage", bufs=4) as stp,
            tc.tile_pool(name="pmean", bufs=2, space="PSUM") as pmean,
            tc.tile_pool(name="pdense", bufs=2, space="PSUM") as pdense,
            tc.tile_pool(name="ptr", bufs=1, space="PSUM") as ptr,
            tc.tile_pool(name="dram", bufs=1, space="DRAM") as dr,
        ):
            # ---- constants. Small ones are funneled through a DVE copy so
            # consumers carry few waits; big ones are DMA'd directly.
            _cid = [0]
            def load_const(shape, dt, src_ap):
                _cid[0] += 1
                ld = cst.tile(shape, dt, tag=f"cld{_cid[0]}", name=f"cld{_cid[0]}")
                nc.sync.dma_start(ld[:], src_ap)
                t = cst.tile(shape, dt, tag=f"cst{_cid[0]}", name=f"cst{_cid[0]}")
                nc.vector.tensor_copy(t[:], ld[:])
                return t

            def load_direct(shape, dt, src_ap, name):
                t = cst.tile(shape, dt, tag=name, name=name)
                nc.sync.dma_start(t[:], src_ap)
                return t

            iota_sb = load_const([P, P], bf16, iota_d[:])
            dloc_sb = load_const([P, NCH], bf16, dloc_d[:])
            invd_sb = load_direct([P, NPAD], bf16, invd_d[:], "invd_sb")
            if GVAR in ("gant", "gant2"):
                gidx_sb = load_direct([P, NCH * 8], mybir.dt.int16, gidx_d[:],
                                      "gidx_sb")
            elif GVAR == "ind":
                idx_sb = load_direct([P, NCH], mybir.dt.int32, idx32_d[:],
                                     "idx_sb")
            else:
                # batched: 256 B half-row indices (512 B rows hit a SWDGE
                # sub-descriptor granularity bug with multi-column offsets)
                idxh_sb = load_direct([P, 2 * NCH], mybir.dt.int32,
                                      idx32h_d[:], "idxh_sb")
            bl_sb = load_const([P, 6], f32, bl_d[:])
            wo_sb = [load_const([P, OUT], bf16, wo_d[h * P:(h + 1) * P, :])
                     for h in range(HID // P)]
            # bo broadcast to 128 partitions via DMA step-0
            bo_ld = cst.tile([P, OUT], f32)
            nc.sync.dma_start(bo_ld[:], bo_d[0:1, :].to_broadcast([P, OUT]))
            bo_sb = cst.tile([P, OUT], f32)
            nc.vector.tensor_copy(bo_sb[:], bo_ld[:])
            wl_sb = {}
            wr_sb = {}
            for l, cin in ((1, C1), (2, HID), (3, HID)):
                wl_sb[l] = [load_const([P, HID], bf16, wl_d[l][h * P:(h + 1) * P, :])
                            for h in range(cin // P)]
                wr_sb[l] = [load_const([P, HID], bf16, wr_d[l][h * P:(h + 1) * P, :])
                            for h in range(cin // P)]
            identity = cst.tile([P, P], bf16)
            from concourse.masks import make_identity
            make_identity(nc, identity[:])

            # ---- feature buffers (transposed layout, [128, NPAD] per half)
            xT_sb = [featp.tile([P, NPAD], bf16, tag=f"ft0_{h}", name=f"xT_sb{h}") for h in range(KH)]
            hT_sb = [featp.tile([P, NPAD], bf16, tag=f"ft1_{h}", name=f"hT_sb{h}") for h in range(KH)]
            meanT_sb = [featp.tile([P, NPAD], bf16, tag=f"mt_{h}", name=f"meanT_sb{h}") for h in range(KH)]
            nc.sync.dma_start(xT_sb[0][:], xT_d[:])

            # DRAM halo buffers (per-layer Shared outputs for the AllGather)
            h_own = dr.tile([NP, HID], bf16)
            h_fulls = [dr.tile([N, HID], bf16, name=f"h_full{i}", tag=f"hf{i}")
                       for i in range(2)]

            def build_oh(t):
                """Binary one-hot block for node-tile t: [128, m_t*128] bf16."""
                m = mtot[t]
                c0 = choff[t]
                oh = ohp.tile([P, MMAX * P], bf16, tag="oh")
                nc.vector.tensor_tensor(
                    out=oh[:, :m * P].rearrange("p (m n) -> p m n", m=m),
                    in0=dloc_sb[:, c0:c0 + m][:, :, None].to_broadcast([P, m, P]),
                    in1=iota_sb[:, None, :].to_broadcast([P, m, P]),
                    op=mybir.AluOpType.is_equal)
                return oh

            def aggregate(layer, cin):
                """meanT_sb <- segment-mean of gathered source features."""
                khalves = cin // P
                if "agg" in ABLATE:
                    return
                for t in range(TILES):
                    m = mtot[t]
                    c0 = choff[t]
                    oh = build_oh(t)
                    pm = [pmean.tile([P, P], f32, tag=f"pm{h}", space="PSUM",
                                     name=f"pm_{t}_{h}") for h in range(khalves)]
                    if layer == 1:
                        for mb_ in range(0, m, KB):
                            nb = min(KB, m - mb_)
                            g = gp.tile([P, KB * C1], bf16, tag="g1")
                            nc.scalar.dma_start(
                                g[:, :nb * C1],
                                xg1_d[:, c0 + mb_:c0 + mb_ + nb, :])
                            for j in range(nb):
                                mm = mb_ + j
                                nc.tensor.matmul(
                                    pm[0][:], lhsT=g[:, j * C1:(j + 1) * C1],
                                    rhs=oh[:, mm * P:(mm + 1) * P],
                                    start=(mm == 0), stop=(mm == m - 1))
                    elif GVAR.startswith("b"):
                        # batched indirect gather, KBATCH chunks per call:
                        # gb[p, j*HID:(j+1)*HID] = h_full[idx[p, c0+cb+j], :]
                        if GVAR == "bind":
                            kbatch, gtag, gpool = m, "gb", gbp
                        elif GVAR == "bind1":
                            kbatch, gtag, gpool = m, "gb1", gb1p
                        else:
                            kbatch, gtag, gpool = int(GVAR[1:]), "gbk", gbp
                        h_full = h_fulls[layer - 2]
                        gsz = MMAX if GVAR in ("bind", "bind1") else kbatch
                        h_half = h_full[:].rearrange("r (h c) -> (r h) c", h=2)
                        for cb in range(0, m, kbatch):
                            nb = min(kbatch, m - cb)
                            gb = gpool.tile([P, gsz * HID], bf16, tag=gtag)
                            nc.gpsimd.indirect_dma_start(
                                out=gb[:, :nb * HID], out_offset=None,
                                in_=h_half,
                                in_offset=bass.IndirectOffsetOnAxis(
                                    ap=idxh_sb[:, 2 * (c0 + cb):2 * (c0 + cb + nb)],
                                    axis=0))
                            for j in range(nb):
                                mm = cb + j
                                for h in range(khalves):
                                    nc.tensor.matmul(
                                        pm[h][:],
                                        lhsT=gb[:, j * HID + h * P:j * HID + (h + 1) * P],
                                        rhs=oh[:, mm * P:(mm + 1) * P],
                                        start=(mm == 0), stop=(mm == m - 1))
                    elif GVAR in ("gant", "gant2"):
                        # dma_gather calls of at most 1024 indices (8 chunks):
                        # larger calls overflow the SWDGE descriptor ring and
                        # crash the exec unit.
                        KC = 8
                        h_full = h_fulls[layer - 2]
                        blocks = []
                        if mlos[t]:
                            blocks.append((0, mlos[t], h_full[0:SPLIT, :]))
                        if mhis[t]:
                            blocks.append((mlos[t], mhis[t], h_full[SPLIT:N, :]))
                        for coff, mb_, src_ap in blocks:
                            for cb in range(0, mb_, KC):
                                nb = min(KC, mb_ - cb)
                                j0 = coff + cb
                                gb = gbp.tile([P, KC, HID], bf16, tag="gbk")
                                nc.gpsimd.dma_gather(
                                    out_ap=gb[:, :nb, :], in_ap=src_ap,
                                    idxs_ap=gidx_sb[:, (c0 + j0) * 8:
                                                    (c0 + j0 + nb) * 8],
                                    num_idxs=nb * P, num_idxs_reg=nb * P,
                                    elem_size=HID)
                                for j in range(nb):
                                    mm = j0 + j
                                    for h in range(khalves):
                                        nc.tensor.matmul(
                                            pm[h][:],
                                            lhsT=gb[:, j, h * P:(h + 1) * P],
                                            rhs=oh[:, mm * P:(mm + 1) * P],
                                            start=(mm == 0), stop=(mm == m - 1))
                    else:
                        h_full = h_fulls[layer - 2]
                        for mm in range(m):
                            k = c0 + mm
                            g = gp.tile([P, HID], bf16, tag="g2")
                            nc.gpsimd.indirect_dma_start(
                                out=g[:], out_offset=None, in_=h_full[:],
                                in_offset=bass.IndirectOffsetOnAxis(
                                    ap=idx_sb[:, k:k + 1], axis=0))
                            for h in range(khalves):
                                nc.tensor.matmul(
                                    pm[h][:], lhsT=g[:, h * P:(h + 1) * P],
                                    rhs=oh[:, mm * P:(mm + 1) * P],
                                    start=(mm == 0), stop=(mm == m - 1))
                    for h in range(khalves):
                        nc.vector.tensor_tensor(
                            out=meanT_sb[h][:, t * P:(t + 1) * P],
                            in0=pm[h][:], in1=invd_sb[:, t * P:(t + 1) * P],
                            op=mybir.AluOpType.mult)

            def dense(layer, cin, src_feat, dst_feat):
                """dst_feat[j] = relu(Wl.T @ meanT + Wr.T @ src_feat + bl)."""
                khalves = cin // P
                if "dense" in ABLATE:
                    return
                for j in range(KH):
                    for b0, blen in nblocks:
                        pd = pdense.tile([P, 512], f32, tag="pd", space="PSUM")
                        nmm = 2 * khalves
                        i = 0
                        for h in range(khalves):
                            nc.tensor.matmul(
                                pd[:, :blen],
                                lhsT=wl_sb[layer][h][:, j * P:(j + 1) * P],
                                rhs=meanT_sb[h][:, b0:b0 + blen],
                                start=(i == 0), stop=(i == nmm - 1)); i += 1
                            nc.tensor.matmul(
                                pd[:, :blen],
                                lhsT=wr_sb[layer][h][:, j * P:(j + 1) * P],
                                rhs=src_feat[h][:, b0:b0 + blen],
                                start=(i == 0), stop=(i == nmm - 1)); i += 1
                        nc.vector.tensor_scalar(
                            out=dst_feat[j][:, b0:b0 + blen], in0=pd[:, :blen],
                            scalar1=bl_sb[:, 2 * (layer - 1) + j:2 * (layer - 1) + j + 1],
                            scalar2=0.0,
                            op0=mybir.AluOpType.add, op1=mybir.AluOpType.max)

            def write_rows(feat, layer):
                """Transpose hT -> row-major h_own, then AllGather into h_fulls."""
                if "rows" in ABLATE:
                    return
                for t in range(TILES):
                    rows = stp.tile([P, HID], bf16, tag="rows")
                    for j in range(KH):
                        pt = ptr.tile([P, P], bf16, tag="pt", space="PSUM")
                        nc.tensor.transpose(
                            pt[:], feat[j][:, t * P:(t + 1) * P], identity[:])
                        nc.vector.tensor_copy(rows[:, j * P:(j + 1) * P], pt[:])
                    nrow = min(P, NP - t * P)
                    nc.scalar.dma_start(h_own[t * P:t * P + nrow, :], rows[:nrow, :])
                if "coll" not in ABLATE:
                    nc.gpsimd.collective_compute(
                        "AllGather", mybir.AluOpType.bypass,
                        replica_groups=[list(range(NCORES))],
                        ins=[h_own[:]], outs=[h_fulls[layer - 1][:]])

            # ---- layer 1
            with nc.named_scope("agg1"):
                aggregate(1, C1)
            with nc.named_scope("dense1"):
                dense(1, C1, xT_sb, hT_sb)
            with nc.named_scope("rows1"):
                write_rows(hT_sb, layer=1)
            # ---- layer 2
            with nc.named_scope("agg2"):
                aggregate(2, HID)
            with nc.named_scope("dense2"):
                dense(2, HID, hT_sb, xT_sb)   # ping-pong: xT_sb now holds h2T
            with nc.named_scope("rows2"):
                write_rows(xT_sb, layer=2)
            # ---- layer 3
            with nc.named_scope("agg3"):
                aggregate(3, HID)
            with nc.named_scope("dense3"):
                dense(3, HID, xT_sb, hT_sb)   # hT_sb now holds h3T
            # ---- output layer: out[n, :] = h3.T @ Wo + bo
            for t in range(TILES):
                po = ptr.tile([P, OUT], f32, tag="po", space="PSUM")
                for h in range(KH):
                    nc.tensor.matmul(
                        po[:], lhsT=hT_sb[h][:, t * P:(t + 1) * P],
                        rhs=wo_sb[h][:],
                        start=(h == 0), stop=(h == KH - 1))
                orow = stp.tile([P, OUT], f32, tag="orow")
                nc.vector.tensor_tensor(out=orow[:], in0=po[:], in1=bo_sb[:],
                                        op=mybir.AluOpType.add)
                nrow = min(P, NP - t * P)
                nc.sync.dma_start(out_d[t * P:t * P + nrow, :], orow[:nrow, :])
    return nc


_PROGRAM_CACHE = {}


def _get_program(mlos, mhis, gvar=None):
    gv = gvar if gvar is not None else GVAR
    key = (mlos, mhis, gv)
    if key not in _PROGRAM_CACHE:
        nc = bacc.Bacc("TRN2", target_bir_lowering=False, debug=False,
                       num_devices=NCORES)
        _build(nc, mlos, mhis, gvar=gv)
        nc.compile()
        _PROGRAM_CACHE[key] = nc
    return _PROGRAM_CACHE[key]


def make_in_maps(inputs):
    x = np.asarray(inputs["x"], np.float32)
    per_core, mlos, mhis = _preprocess(x, np.asarray(inputs["edge_index"]))
    iota = np.tile(np.arange(P, dtype=np.float32)[None, :], (P, 1)).astype(_bf)
    bl = np.zeros((P, 6), np.float32)
    for l in (1, 2, 3):
        b = np.asarray(inputs[f"bl{l}"], np.float32)
        bl[:, 2 * (l - 1)] = b[:P]
        bl[:, 2 * (l - 1) + 1] = b[P:]
    common = {"iota": iota, "bl": bl,
              "Wo": np.asarray(inputs["Wo"]).astype(_bf),
              "bo": np.asarray(inputs["bo"], np.float32).reshape(1, OUT)}
    for l in (1, 2, 3):
        common[f"Wl{l}"] = np.asarray(inputs[f"Wl{l}"]).astype(_bf)
        common[f"Wr{l}"] = np.asarray(inputs[f"Wr{l}"]).astype(_bf)
    in_maps = []
    for d in range(NCORES):
        pc = per_core[d]
        in_maps.append({**common, "gidx": pc["gidx"], "idx32": pc["idx32"],
                        "idx32h": pc["idx32h"], "dloc": pc["dloc"],
                        "invd": pc["invd"], "xg1": pc["xg1"], "xT": pc["xT"]})
    return in_maps, mlos, mhis


def kernel(**inputs) -> np.ndarray:
    in_maps, mlos, mhis = make_in_maps(inputs)
    nc = _get_program(mlos, mhis)
    res = run_bass_kernel_spmd(nc, in_maps, core_ids=list(range(NCORES)))
    out = np.concatenate(
        [np.asarray(res.results[d]["out"], np.float32) for d in range(NCORES)], axis=0)
    return out


# revision 37
# speedup vs baseline: 1.1423x; 1.1423x over previous
"""GraphSAGE-style 3-layer GNN (mean aggregation) on 8 Trainium2 NeuronCores.

Strategy (dst-sharded graph parallelism):
- Nodes (and their incoming edges) are sharded across 8 cores: core d owns
  dst nodes [d*6250, (d+1)*6250).
- Host preprocessing sorts each core's edges by (dst node-tile, src>=SPLIT,
  src) and packs them into 128-edge chunks. Per tile t the chunk counts
  (Mlo[t], Mhi[t]) are maxed over cores so all 8 cores share one program;
  pad slots get dloc=200 (matches no one-hot column, so they contribute 0).
- Aggregation on device: per tile a binary one-hot block
  OH[e, n] = (dloc[e] == n) is built on the DVE from an iota tile, and
  sumT[c, n] += G[e, c].T @ OH[e, n] accumulates in PSUM on the PE. The
  1/deg mean scaling is folded into the PSUM->SBUF copy (tensor_tensor mult
  with a replicated inv-degree row).
- Layer-1 edge source features are pre-gathered on the host and streamed
  sequentially; layers 2/3 gather their source features from a replicated
  DRAM table with batched gpsimd dma_gather calls of at most 1024 indices
  (8 chunks) each — larger calls overflow the SWDGE descriptor ring and
  crash the exec unit. Indices are int16 relative to a src<SPLIT or
  src>=SPLIT base (the lo/hi edge split exists for this).
- Plain multi-column indirect_dma_start batching is NOT usable: the HW
  SWDGE ucode only honors one offset per partition and walks contiguously
  (CoreSim models proper per-element semantics — they diverge).
- The replicated table is refreshed between layers with an AllGather
  (halo exchange) over the 8 cores into per-layer internal DRAM buffers.
- Dense transforms run in transposed layout: hT = Wl.T @ meanT + Wr.T @ xT,
  bias add + ReLU on the DVE, then PE-transposes produce row-major h for the
  next layer's gather table.
Compute dtype: bf16 inputs with fp32 PSUM accumulation.
"""
import os
import numpy as np
import ml_dtypes

import concourse.bass as bass
import concourse.bacc as bacc
import concourse.mybir as mybir
import concourse.tile as tile
from concourse.bass_utils import run_bass_kernel_spmd

N = 50000
E = 800000
C1 = 128
HID = 256
OUT = 15
NCORES = 8
NP = N // NCORES          # 6250 own nodes per core
P = 128
TILES = (NP + P - 1) // P  # 49 node tiles per core
NPAD = TILES * P           # 6272
KB = 6                     # L1 staged-gather chunks per DMA
SPLIT = 25088              # low/high src base split for int16 gather indices

bf16 = mybir.dt.bfloat16
f32 = mybir.dt.float32
_bf = ml_dtypes.bfloat16
ABLATE = set(os.environ.get("ABLATE", "").split(",")) - {""}  # timing experiments
GVAR = os.environ.get("GVAR", "gant2")  # "gant2" | "bind" | "gant" | "ind"


def _preprocess(x, edge_index):
    """Sort/pad edges into per-tile (lo, hi) chunk blocks shared across cores."""
    src = np.ascontiguousarray(edge_index[0]).astype(np.int64)
    dst = np.ascontiguousarray(edge_index[1]).astype(np.int64)
    deg = np.bincount(dst, minlength=N)
    invdeg = (1.0 / np.maximum(deg, 1)).astype(np.float32)

    core = dst // NP
    tilei = (dst - core * NP) // P
    ishi = (src >= SPLIT).astype(np.int64)
    gkey = (core * TILES + tilei) * 2 + ishi
    order = np.lexsort((src, gkey))
    sg = gkey[order]
    ssrc = src[order]
    sdst = dst[order]
    NG = NCORES * TILES * 2
    starts = np.searchsorted(sg, np.arange(NG))
    counts = np.diff(np.append(starts, E)).reshape(NCORES, TILES, 2)
    mlo = np.ceil(counts[:, :, 0] / P).astype(np.int64).max(axis=0)  # [TILES]
    mhi = np.ceil(counts[:, :, 1] / P).astype(np.int64).max(axis=0)
    mtot = mlo + mhi
    assert mtot.min() > 0
    chunkoff = np.zeros(TILES + 1, np.int64)
    chunkoff[1:] = np.cumsum(mtot)
    NCH = int(chunkoff[-1])

    # slot base for each (tile, lo/hi) block, per core (same for all cores)
    blk_slot0 = np.zeros((TILES, 2), np.int64)
    blk_slot0[:, 0] = chunkoff[:-1] * P
    blk_slot0[:, 1] = (chunkoff[:-1] + mlo) * P

    # per-core flattened slot grids [NCH*128]
    rank = np.arange(E) - starts[sg]
    tile_of = (sg // 2) % TILES
    hi_of = sg % 2
    slot_in_core = blk_slot0[tile_of, hi_of] + rank
    core_of = sg // (2 * TILES)

    srcg = np.zeros((NCORES, NCH * P), np.int64)
    # pad defaults: lo blocks gather row 0, hi blocks gather row SPLIT
    for t in range(TILES):
        srcg[:, blk_slot0[t, 1]:blk_slot0[t, 1] + mhi[t] * P] = SPLIT
    dlocg = np.full((NCORES, NCH * P), 200.0, np.float32)
    srcg[core_of, slot_in_core] = ssrc
    dlocg[core_of, slot_in_core] = sdst - core_of * NP - tile_of * P

    x_bf = x.astype(_bf)
    rel = srcg - (srcg >= SPLIT) * SPLIT
    assert rel.max() < 32768
    per_core = []
    for d in range(NCORES):
        # int16 gather indices packed [16, n/16] per call block, replicated x8
        relc = rel[d].astype(np.int16).reshape(NCH, P)
        packed = relc.reshape(NCH * 8, 16).T            # [16, NCH*8]
        gidx = np.ascontiguousarray(np.tile(packed, (8, 1)))  # [128, NCH*8]
        idx32 = np.ascontiguousarray(
            srcg[d].reshape(NCH, P).T).astype(np.int32)  # [128, NCH]
        # half-row (256 B) gather indices: pairs (2r, 2r+1) per edge
        idx32h = np.empty((P, 2 * NCH), np.int32)
        idx32h[:, 0::2] = 2 * idx32
        idx32h[:, 1::2] = 2 * idx32 + 1
        dloc = np.ascontiguousarray(
            dlocg[d].reshape(NCH, P).T).astype(_bf)      # [128, NCH]
        invd = np.zeros(NPAD, np.float32)
        invd[:NP] = invdeg[d * NP:(d + 1) * NP]
        invd = np.ascontiguousarray(
            np.tile(invd[None, :], (P, 1))).astype(_bf)  # [128, NPAD]
        # L1 pre-gathered edge features [128, NCH, C1]
        xg1 = np.ascontiguousarray(
            x_bf[srcg[d].reshape(NCH, P)].transpose(1, 0, 2))
        # own transposed features [128, NPAD]
        xT = np.zeros((C1, NPAD), np.float32)
        xT[:, :NP] = x[d * NP:(d + 1) * NP].T
        per_core.append(dict(gidx=gidx, idx32=idx32, idx32h=idx32h, dloc=dloc,
                             invd=invd, xg1=xg1,
                             xT=np.ascontiguousarray(xT).astype(_bf)))
    return per_core, tuple(int(v) for v in mlo), tuple(int(v) for v in mhi)


def _build(nc: bass.Bass, mlos, mhis, gvar=None):
    GVAR = gvar if gvar is not None else globals()["GVAR"]
    mtot = [a + b for a, b in zip(mlos, mhis)]
    choff = [0]
    for m in mtot:
        choff.append(choff[-1] + m)
    NCH = choff[-1]
    MMAX = max(mtot)
    # ---- I/O ----
    gidx_d = nc.dram_tensor("gidx", [P, NCH * 8], mybir.dt.int16, kind="ExternalInput")
    idx32_d = nc.dram_tensor("idx32", [P, NCH], mybir.dt.int32, kind="ExternalInput")
    idx32h_d = nc.dram_tensor("idx32h", [P, 2 * NCH], mybir.dt.int32,
                              kind="ExternalInput")
    dloc_d = nc.dram_tensor("dloc", [P, NCH], bf16, kind="ExternalInput")
    invd_d = nc.dram_tensor("invd", [P, NPAD], bf16, kind="ExternalInput")
    xg1_d = nc.dram_tensor("xg1", [P, NCH, C1], bf16, kind="ExternalInput")
    xT_d = nc.dram_tensor("xT", [P, NPAD], bf16, kind="ExternalInput")
    iota_d = nc.dram_tensor("iota", [P, P], bf16, kind="ExternalInput")
    wl_d = {}
    wr_d = {}
    for l, cin in ((1, C1), (2, HID), (3, HID)):
        wl_d[l] = nc.dram_tensor(f"Wl{l}", [cin, HID], bf16, kind="ExternalInput")
        wr_d[l] = nc.dram_tensor(f"Wr{l}", [cin, HID], bf16, kind="ExternalInput")
    bl_d = nc.dram_tensor("bl", [P, 6], f32, kind="ExternalInput")       # [:, 2(l-1)+j]
    wo_d = nc.dram_tensor("Wo", [HID, OUT], bf16, kind="ExternalInput")
    bo_d = nc.dram_tensor("bo", [1, OUT], f32, kind="ExternalInput")
    out_d = nc.dram_tensor("out", [NP, OUT], f32, kind="ExternalOutput")

    KH = HID // P  # 2 halves of the hidden dim
    nblocks = [(b, min(512, NPAD - b)) for b in range(0, NPAD, 512)]

    with tile.TileContext(nc) as tc:
        with (
            tc.tile_pool(name="const", bufs=1) as cst,
            tc.tile_pool(name="feat", bufs=1) as featp,
            tc.tile_pool(name="g", bufs=24) as gp,
            tc.tile_pool(name="gb", bufs=3) as gbp,
            tc.tile_pool(name="gb1", bufs=1) as gb1p,
            tc.tile_pool(name="oh", bufs=4) as ohp,
            tc.tile_pool(name="stage", bufs=4) as stp,
            tc.tile_pool(name="pmean", bufs=2, space="PSUM") as pmean,
            tc.tile_pool(name="pdense", bufs=2, space="PSUM") as pdense,
            tc.tile_pool(name="ptr", bufs=1, space="PSUM") as ptr,
            tc.tile_pool(name="dram", bufs=1, space="DRAM") as dr,
        ):
            # ---- constants. Small ones are funneled through a DVE copy so
            # consumers carry few waits; big ones are DMA'd directly.
            _cid = [0]
            def load_const(shape, dt, src_ap):
                _cid[0] += 1
                ld = cst.tile(shape, dt, tag=f"cld{_cid[0]}", name=f"cld{_cid[0]}")
                nc.sync.dma_start(ld[:], src_ap)
                t = cst.tile(shape, dt, tag=f"cst{_cid[0]}", name=f"cst{_cid[0]}")
                nc.vector.tensor_copy(t[:], ld[:])
                return t

            def load_direct(shape, dt, src_ap, name):
                t = cst.tile(shape, dt, tag=name, name=name)
                nc.sync.dma_start(t[:], src_ap)
                return t

            iota_sb = load_const([P, P], bf16, iota_d[:])
            dloc_sb = load_const([P, NCH], bf16, dloc_d[:])
            invd_sb = load_direct([P, NPAD], bf16, invd_d[:], "invd_sb")
            if GVAR.startswith("gant"):
                gidx_sb = load_direct([P, NCH * 8], mybir.dt.int16, gidx_d[:],
                                      "gidx_sb")
            elif GVAR == "ind":
                idx_sb = load_direct([P, NCH], mybir.dt.int32, idx32_d[:],
                                     "idx_sb")
            else:
                # batched: 256 B half-row indices (512 B rows hit a SWDGE
                # sub-descriptor granularity bug with multi-column offsets)
                idxh_sb = load_direct([P, 2 * NCH], mybir.dt.int32,
                                      idx32h_d[:], "idxh_sb")
            bl_sb = load_const([P, 6], f32, bl_d[:])
            wo_sb = [load_const([P, OUT], bf16, wo_d[h * P:(h + 1) * P, :])
                     for h in range(HID // P)]
            # bo broadcast to 128 partitions via DMA step-0
            bo_ld = cst.tile([P, OUT], f32)
            nc.sync.dma_start(bo_ld[:], bo_d[0:1, :].to_broadcast([P, OUT]))
            bo_sb = cst.tile([P, OUT], f32)
            nc.vector.tensor_copy(bo_sb[:], bo_ld[:])
            wl_sb = {}
            wr_sb = {}
            for l, cin in ((1, C1), (2, HID), (3, HID)):
                wl_sb[l] = [load_const([P, HID], bf16, wl_d[l][h * P:(h + 1) * P, :])
                            for h in range(cin // P)]
                wr_sb[l] = [load_const([P, HID], bf16, wr_d[l][h * P:(h + 1) * P, :])
                            for h in range(cin // P)]
            identity = cst.tile([P, P], bf16)
            from concourse.masks import make_identity
            make_identity(nc, identity[:])

            # ---- feature buffers (transposed layout, [128, NPAD] per half)
            xT_sb = [featp.tile([P, NPAD], bf16, tag=f"ft0_{h}", name=f"xT_sb{h}") for h in range(KH)]
            hT_sb = [featp.tile([P, NPAD], bf16, tag=f"ft1_{h}", name=f"hT_sb{h}") for h in range(KH)]
            meanT_sb = [featp.tile([P, NPAD], bf16, tag=f"mt_{h}", name=f"meanT_sb{h}") for h in range(KH)]
            nc.sync.dma_start(xT_sb[0][:], xT_d[:])

            # DRAM halo buffers (per-layer Shared outputs for the AllGather)
            h_own = dr.tile([NP, HID], bf16)
            h_fulls = [dr.tile([N, HID], bf16, name=f"h_full{i}", tag=f"hf{i}")
                       for i in range(2)]

            def build_oh(t):
                """Binary one-hot block for node-tile t: [128, m_t*128] bf16."""
                m = mtot[t]
                c0 = choff[t]
                oh = ohp.tile([P, MMAX * P], bf16, tag="oh")
                nc.vector.tensor_tensor(
                    out=oh[:, :m * P].rearrange("p (m n) -> p m n", m=m),
                    in0=dloc_sb[:, c0:c0 + m][:, :, None].to_broadcast([P, m, P]),
                    in1=iota_sb[:, None, :].to_broadcast([P, m, P]),
                    op=mybir.AluOpType.is_equal)
                return oh

            _gq = [0]  # gather-call counter for queue alternation (gant3)

            def aggregate(layer, cin):
                """meanT_sb <- segment-mean of gathered source features."""
                khalves = cin // P
                if "agg" in ABLATE:
                    return
                for t in range(TILES):
                    m = mtot[t]
                    c0 = choff[t]
                    oh = build_oh(t)
                    pm = [pmean.tile([P, P], f32, tag=f"pm{h}", space="PSUM",
                                     name=f"pm_{t}_{h}") for h in range(khalves)]
                    if layer == 1:
                        for mb_ in range(0, m, KB):
                            nb = min(KB, m - mb_)
                            g = gp.tile([P, KB * C1], bf16, tag="g1")
                            nc.scalar.dma_start(
                                g[:, :nb * C1],
                                xg1_d[:, c0 + mb_:c0 + mb_ + nb, :])
                            for j in range(nb):
                                mm = mb_ + j
                                nc.tensor.matmul(
                                    pm[0][:], lhsT=g[:, j * C1:(j + 1) * C1],
                                    rhs=oh[:, mm * P:(mm + 1) * P],
                                    start=(mm == 0), stop=(mm == m - 1))
                    elif GVAR.startswith("b"):
                        # batched indirect gather, KBATCH chunks per call:
                        # gb[p, j*HID:(j+1)*HID] = h_full[idx[p, c0+cb+j], :]
                        if GVAR == "bind":
                            kbatch, gtag, gpool = m, "gb", gbp
                        elif GVAR == "bind1":
                            kbatch, gtag, gpool = m, "gb1", gb1p
                        else:
                            kbatch, gtag, gpool = int(GVAR[1:]), "gbk", gbp
                        h_full = h_fulls[layer - 2]
                        gsz = MMAX if GVAR in ("bind", "bind1") else kbatch
                        h_half = h_full[:].rearrange("r (h c) -> (r h) c", h=2)
                        for cb in range(0, m, kbatch):
                            nb = min(kbatch, m - cb)
                            gb = gpool.tile([P, gsz * HID], bf16, tag=gtag)
                            nc.gpsimd.indirect_dma_start(
                                out=gb[:, :nb * HID], out_offset=None,
                                in_=h_half,
                                in_offset=bass.IndirectOffsetOnAxis(
                                    ap=idxh_sb[:, 2 * (c0 + cb):2 * (c0 + cb + nb)],
                                    axis=0))
                            for j in range(nb):
                                mm = cb + j
                                for h in range(khalves):
                                    nc.tensor.matmul(
                                        pm[h][:],
                                        lhsT=gb[:, j * HID + h * P:j * HID + (h + 1) * P],
                                        rhs=oh[:, mm * P:(mm + 1) * P],
                                        start=(mm == 0), stop=(mm == m - 1))
                    elif GVAR in ("gant", "gant2", "gant3"):
                        # dma_gather calls of at most 1024 indices (8 chunks):
                        # larger calls overflow the SWDGE descriptor ring and
                        # crash the exec unit.
                        KC = 8
                        h_full = h_fulls[layer - 2]
                        blocks = []
                        if mlos[t]:
                            blocks.append((0, mlos[t], h_full[0:SPLIT, :]))
                        if mhis[t]:
                            blocks.append((mlos[t], mhis[t], h_full[SPLIT:N, :]))
                        for coff, mb_, src_ap in blocks:
                            for cb in range(0, mb_, KC):
                                nb = min(KC, mb_ - cb)
                                j0 = coff + cb
                                gb = gbp.tile([P, KC, HID], bf16, tag="gbk")
                                qn = (_gq[0] % 2) if GVAR == "gant3" else 0
                                _gq[0] += 1
                                nc.gpsimd.dma_gather(
                                    out_ap=gb[:, :nb, :], in_ap=src_ap,
                                    idxs_ap=gidx_sb[:, (c0 + j0) * 8:
                                                    (c0 + j0 + nb) * 8],
                                    num_idxs=nb * P, num_idxs_reg=nb * P,
                                    elem_size=HID, queue_num=qn)
                                for j in range(nb):
                                    mm = j0 + j
                                    for h in range(khalves):
                                        nc.tensor.matmul(
                                            pm[h][:],
                                            lhsT=gb[:, j, h * P:(h + 1) * P],
                                            rhs=oh[:, mm * P:(mm + 1) * P],
                                            start=(mm == 0), stop=(mm == m - 1))
                    else:
                        h_full = h_fulls[layer - 2]
                        for mm in range(m):
                            k = c0 + mm
                            g = gp.tile([P, HID], bf16, tag="g2")
                            nc.gpsimd.indirect_dma_start(
                                out=g[:], out_offset=None, in_=h_full[:],
                                in_offset=bass.IndirectOffsetOnAxis(
                                    ap=idx_sb[:, k:k + 1], axis=0))
                            for h in range(khalves):
                                nc.tensor.matmul(
                                    pm[h][:], lhsT=g[:, h * P:(h + 1) * P],
                                    rhs=oh[:, mm * P:(mm + 1) * P],
                                    start=(mm == 0), stop=(mm == m - 1))
                    for h in range(khalves):
                        nc.vector.tensor_tensor(
                            out=meanT_sb[h][:, t * P:(t + 1) * P],
                            in0=pm[h][:], in1=invd_sb[:, t * P:(t + 1) * P],
                            op=mybir.AluOpType.mult)

            def dense(layer, cin, src_feat, dst_feat):
                """dst_feat[j] = relu(Wl.T @ meanT + Wr.T @ src_feat + bl)."""
                khalves = cin // P
                if "dense" in ABLATE:
                    return
                for j in range(KH):
                    for b0, blen in nblocks:
                        pd = pdense.tile([P, 512], f32, tag="pd", space="PSUM")
                        nmm = 2 * khalves
                        i = 0
                        for h in range(khalves):
                            nc.tensor.matmul(
                                pd[:, :blen],
                                lhsT=wl_sb[layer][h][:, j * P:(j + 1) * P],
                                rhs=meanT_sb[h][:, b0:b0 + blen],
                                start=(i == 0), stop=(i == nmm - 1)); i += 1
                            nc.tensor.matmul(
                                pd[:, :blen],
                                lhsT=wr_sb[layer][h][:, j * P:(j + 1) * P],
                                rhs=src_feat[h][:, b0:b0 + blen],
                                start=(i == 0), stop=(i == nmm - 1)); i += 1
                        nc.vector.tensor_scalar(
                            out=dst_feat[j][:, b0:b0 + blen], in0=pd[:, :blen],
                            scalar1=bl_sb[:, 2 * (layer - 1) + j:2 * (layer - 1) + j + 1],
                            scalar2=0.0,
                            op0=mybir.AluOpType.add, op1=mybir.AluOpType.max)

            def write_rows(feat, layer):
                """Transpose hT -> row-major h_own, then AllGather into h_fulls."""
                if "rows" in ABLATE:
                    return
                for t in range(TILES):
                    rows = stp.tile([P, HID], bf16, tag="rows")
                    for j in range(KH):
                        pt = ptr.tile([P, P], bf16, tag="pt", space="PSUM")
                        nc.tensor.transpose(
                            pt[:], feat[j][:, t * P:(t + 1) * P], identity[:])
                        nc.vector.tensor_copy(rows[:, j * P:(j + 1) * P], pt[:])
                    nrow = min(P, NP - t * P)
                    nc.scalar.dma_start(h_own[t * P:t * P + nrow, :], rows[:nrow, :])
                if "coll" not in ABLATE:
                    nc.gpsimd.collective_compute(
                        "AllGather", mybir.AluOpType.bypass,
                        replica_groups=[list(range(NCORES))],
                        ins=[h_own[:]], outs=[h_fulls[layer - 1][:]])

            # ---- layer 1
            with nc.named_scope("agg1"):
                aggregate(1, C1)
            with nc.named_scope("dense1"):
                dense(1, C1, xT_sb, hT_sb)
            with nc.named_scope("rows1"):
                write_rows(hT_sb, layer=1)
            # ---- layer 2
            with nc.named_scope("agg2"):
                aggregate(2, HID)
            with nc.named_scope("dense2"):
                dense(2, HID, hT_sb, xT_sb)   # ping-pong: xT_sb now holds h2T
            with nc.named_scope("rows2"):
                write_rows(xT_sb, layer=2)
            # ---- layer 3
            with nc.named_scope("agg3"):
                aggregate(3, HID)
            with nc.named_scope("dense3"):
                dense(3, HID, xT_sb, hT_sb)   # hT_sb now holds h3T
            # ---- output layer: out[n, :] = h3.T @ Wo + bo
            for t in range(TILES):
                po = ptr.tile([P, OUT], f32, tag="po", space="PSUM")
                for h in range(KH):
                    nc.tensor.matmul(
                        po[:], lhsT=hT_sb[h][:, t * P:(t + 1) * P],
                        rhs=wo_sb[h][:],
                        start=(h == 0), stop=(h == KH - 1))
                orow = stp.tile([P, OUT], f32, tag="orow")
                nc.vector.tensor_tensor(out=orow[:], in0=po[:], in1=bo_sb[:],
                                        op=mybir.AluOpType.add)
                nrow = min(P, NP - t * P)
                nc.sync.dma_start(out_d[t * P:t * P + nrow, :], orow[:nrow, :])
    return nc


_PROGRAM_CACHE = {}


def _get_program(mlos, mhis, gvar=None):
    gv = gvar if gvar is not None else GVAR
    key = (mlos, mhis, gv)
    if key not in _PROGRAM_CACHE:
        nc = bacc.Bacc("TRN2", target_bir_lowering=False, debug=False,
                       num_devices=NCORES)
        _build(nc, mlos, mhis, gvar=gv)
        nc.compile()
        _PROGRAM_CACHE[key] = nc
    return _PROGRAM_CACHE[key]


def make_in_maps(inputs):
    x = np.asarray(inputs["x"], np.float32)
    per_core, mlos, mhis = _preprocess(x, np.asarray(inputs["edge_index"]))
    iota = np.tile(np.arange(P, dtype=np.float32)[None, :], (P, 1)).astype(_bf)
    bl = np.zeros((P, 6), np.float32)
    for l in (1, 2, 3):
        b = np.asarray(inputs[f"bl{l}"], np.float32)
        bl[:, 2 * (l - 1)] = b[:P]
        bl[:, 2 * (l - 1) + 1] = b[P:]
    common = {"iota": iota, "bl": bl,
              "Wo": np.asarray(inputs["Wo"]).astype(_bf),
              "bo": np.asarray(inputs["bo"], np.float32).reshape(1, OUT)}
    for l in (1, 2, 3):
        common[f"Wl{l}"] = np.asarray(inputs[f"Wl{l}"]).astype(_bf)
        common[f"Wr{l}"] = np.asarray(inputs[f"Wr{l}"]).astype(_bf)
    in_maps = []
    for d in range(NCORES):
        pc = per_core[d]
        in_maps.append({**common, "gidx": pc["gidx"], "idx32": pc["idx32"],
                        "idx32h": pc["idx32h"], "dloc": pc["dloc"],
                        "invd": pc["invd"], "xg1": pc["xg1"], "xT": pc["xT"]})
    return in_maps, mlos, mhis


def kernel(**inputs) -> np.ndarray:
    in_maps, mlos, mhis = make_in_maps(inputs)
    nc = _get_program(mlos, mhis)
    res = run_bass_kernel_spmd(nc, in_maps, core_ids=list(range(NCORES)))
    out = np.concatenate(
        [np.asarray(res.results[d]["out"], np.float32) for d in range(NCORES)], axis=0)
    return out
